# revision 32
# baseline (speedup 1.0000x reference)
"""Trainium2 Bass kernel for the Brain3DNetwork LIF spiking network.

Problem: N = 32*128*128 neurons on a 3D grid, 26-neighbor stencil edges in
COO form with weights 2^exp (exp in 0..8), 10 LIF timesteps with a
log-domain weighted gather-scatter (mathematically: per-dst sum of
w * (prev_spike + 1e-8), computed in f32, cast to f16), then LIF dynamics.

Strategy:
  * Shard the L dimension (32 planes) across 8 NeuronCores, 4 planes each.
  * SBUF layout per core: [h=128 partitions, (l_local, w) = 512 free].
  * The COO edge list is a regular 26-offset stencil; host preprocessing
    groups edges by (dl,dh,dw) offset into 26 dense weight planes per core.
  * Per step, per offset: one bf16 elementwise multiply on the Vector engine
    (weights x shifted-spikes view; shifts along l/w are free-dim AP offsets,
    with a 1-element-shifted spike copy to keep all views 4-byte aligned),
    then a shift-matrix matmul on the Tensor engine accumulating the
    h-shifted sum into PSUM in f32 (integer-exact).
  * Since f32(1 + 1e-8) == 1.0 exactly, the reference's w*(prev+1e-8) equals
    (integer spike contribution) + 1e-8*(sum of non-spiking weights); adding
    a precomputed base = 1e-8 * (total incoming weight) instead differs by
    1e-8*I which is always far below f16 rounding granularity.
  * Cross-core halo exchange of boundary spike planes via an 8-core
    AllGather each step; per-core neighbor slices are addressed dynamically
    with partition_id and out-of-range DMAs are skipped (edge cores).
  * The early-exit (prev.sum()==0 -> total=0) is exact at step 0 (handled by
    construction: step 0 skips the stencil entirely) and unreachable
    afterwards (~46% of neurons spike at step 0 from the external drive).
"""

import numpy as np

L, H, W = 32, 128, 128
N = L * H * W
NCORES = 8
SLAB = L // NCORES          # 4 l-planes per core
FD = SLAB * W               # 512 free dim of the working set
PF = 772                    # prevA free dim: 1 pad + 128 halo + 512 local + 128 halo + 3 pad
NSTEPS = 10
E_EDGES = 13192568

TAU, THRESHOLD = 30.0, 0.1
DECAY = float(np.float32(np.exp(-1.0 / TAU)))
GAIN = float(np.float32(TAU * (1.0 - np.exp(-1.0 / TAU))))

OFFSETS = [
    (dl, dh, dw)
    for dl in (-1, 0, 1)
    for dh in (-1, 0, 1)
    for dw in (-1, 0, 1)
    if not (dl == 0 and dh == 0 and dw == 0)
]

_PROG = {}  # cached compiled programs, keyed by repeats


def _build_program(repeats=1):
    import sys
    if "/opt/trn_rl_repo" not in sys.path:
        sys.path.insert(0, "/opt/trn_rl_repo")
    import concourse.bass as bass
    import concourse.mybir as mybir
    import concourse.tile as tile
    from concourse import bacc
    from concourse.bass import ds

    f32 = mybir.dt.float32
    f16 = mybir.dt.float16
    bf16 = mybir.dt.bfloat16
    Alu = mybir.AluOpType

    nc = bacc.Bacc(
        "TRN2",
        target_bir_lowering=False,
        debug=False,
        enable_asserts=False,
        num_devices=NCORES,
    )

    wt_d = nc.dram_tensor("wt", [128, 26 * FD], bf16, kind="ExternalInput")
    smat_d = nc.dram_tensor("smat", [128, 3 * 128], bf16, kind="ExternalInput")
    base_d = nc.dram_tensor("base", [128, FD], f32, kind="ExternalInput")
    ext_d = nc.dram_tensor("ext", [128, FD], f32, kind="ExternalInput")
    osp_d = nc.dram_tensor("out_spikes", [NSTEPS, 128, FD], bf16, kind="ExternalOutput")
    ovl_d = nc.dram_tensor("out_volts", [NSTEPS, 128, FD], f32, kind="ExternalOutput")

    RG = [list(range(NCORES))]

    with tile.TileContext(nc) as tc:
        with (
            tc.tile_pool(name="const", bufs=1) as cpool,
            tc.tile_pool(name="state", bufs=1) as spool,
            tc.tile_pool(name="work", bufs=2) as wpool,
            tc.tile_pool(name="tmpp", bufs=20) as tpool,
            tc.tile_pool(name="psum", bufs=2, space="PSUM") as ppool,
            tc.tile_pool(name="dram", bufs=2, space="DRAM") as dpool,
        ):
            # ext first: step 0 needs only it. Boundary columns arrive first
            # (they gate the first AllGather trigger on every core).
            ext = cpool.tile([128, FD], f32, name="ext_s")
            nc.sync.dma_start(
                ext[:, :].rearrange("p (b w) -> p b w", b=4)[:, ::3, :],
                ext_d[:, :].rearrange("p (b w) -> p b w", b=4)[:, ::3, :],
            )
            nc.scalar.dma_start(ext[:, 128:384], ext_d[:, 128:384])
            smat = cpool.tile([128, 3 * 128], bf16, name="smat_s")
            nc.scalar.dma_start(smat[:], smat_d[:])
            base = cpool.tile([128, FD], f32, name="base_s")
            nc.scalar.dma_start(base[:], base_d[:])
            # weights: split across queues/engines to parallelize the 3.4MB load
            wt = cpool.tile([128, 26 * FD], bf16, name="wt_s")
            NCH = 8
            cw = 26 * FD // NCH
            for j in range(NCH):
                eng = nc.sync if j % 2 == 0 else nc.scalar
                eng.dma_start(wt[:, j * cw : (j + 1) * cw], wt_d[:, j * cw : (j + 1) * cw])

            prevA = [spool.tile([128, PF], bf16, name=f"prevA{i}") for i in range(2)]
            prevB = [spool.tile([128, PF], bf16, name=f"prevB{i}") for i in range(2)]
            vt = [spool.tile([128, FD], f32, name=f"vt{i}") for i in range(2)]
            for i in range(2):
                nc.vector.memset(prevA[i][:], 0.0)
            # persistent dl=0 product tiles; the never-written edge column of
            # each group stays zero from this memset (its weight is zero, and
            # skipping it removes the halo-column read dependency entirely).
            tmpza = [spool.tile([128, FD], bf16, name=f"tmpza{i}") for i in range(2)]
            tmpzb = [spool.tile([128, FD], bf16, name=f"tmpzb{i}") for i in range(2)]
            for z in tmpza + tmpzb:
                nc.vector.memset(z[:], 0.0)

            pid = nc.sync.partition_id()
            pid2 = nc.scalar.partition_id()

            # stencil terms grouped: dl=0 first (full-width, no halo deps),
            # then dl=+-1 split into interior/boundary column ranges.
            terms_dl0 = [(k, o) for k, o in enumerate(OFFSETS) if o[0] == 0]
            terms_dlp = [(k, o) for k, o in enumerate(OFFSETS) if o[0] == 1]
            terms_dlm = [(k, o) for k, o in enumerate(OFFSETS) if o[0] == -1]

            ag_out_prev = None
            for rep in range(repeats):
              for t in range(NSTEPS):
                me = t % 2
                pv = 1 - me

                if t == 0:
                    # boundary-first so the first AllGather staging can launch
                    # before the mid columns are computed
                    vp = wpool.tile([128, FD], f32, name="vp", tag="vp")
                    spk_dst0 = prevA[me][:, 129:641]
                    for vp_v, ext_v, spk_v in (
                        (
                            vp[:, :].rearrange("p (b w) -> p b w", b=4)[:, ::3, :],
                            ext[:, :].rearrange("p (b w) -> p b w", b=4)[:, ::3, :],
                            spk_dst0.rearrange("p (b w) -> p b w", b=4)[:, ::3, :],
                        ),
                        (vp[:, 128:384], ext[:, 128:384], prevA[me][:, 257:513]),
                    ):
                        nc.vector.tensor_scalar(vp_v, ext_v, GAIN, None, op0=Alu.mult)
                        nc.vector.tensor_scalar(
                            spk_v, vp_v, THRESHOLD, None, op0=Alu.is_ge
                        )
                else:
                    acc = ppool.tile([128, FD], f32, name="acc", tag="acc")
                    # zero PSUM so matmuls can accumulate in any order
                    # (no start=True ordering hazard)
                    nc.vector.memset(acc[:], 0.0)
                    # v*decay on the scalar engine, off the critical path
                    vd = wpool.tile([128, FD], f32, name="vd", tag="vd")
                    nc.scalar.mul(vd[:], vt[pv][:], DECAY)
                    # interior part of the 1-element-shifted spike copy
                    # (keeps dw=0 views 4B-aligned); halo strips come later
                    nc.vector.tensor_copy(
                        prevB[pv][:, 130:642], prevA[pv][:, 129:641]
                    )

                    # --- interior stencil work (no halo dependency) ---
                    # dl=0: one edge column per term is out-of-plane (zero
                    # weight); skip it so the mul never reads a halo column.
                    zi_a = zi_b = 0
                    for k, (dl, dh, dw) in terms_dl0:
                        o = -dw
                        if dw == 0:
                            # reads prevB[130:642) == local data only: full width
                            tz = tpool.tile([128, FD], bf16, name="tmp", tag="tmp")
                            wr = slice(0, FD)
                        elif dw < 0:
                            tz = tmpza[zi_a % 2]; zi_a += 1
                            wr = slice(0, FD - 1)      # col FD-1 stays zero
                        else:
                            tz = tmpzb[zi_b % 2]; zi_b += 1
                            wr = slice(1, FD)          # col 0 stays zero
                        if dw == 0:
                            src = prevB[pv][:, 130 + o + wr.start : 130 + o + wr.stop]
                        else:
                            src = prevA[pv][:, 129 + o + wr.start : 129 + o + wr.stop]
                        nc.vector.tensor_mul(tz[:, wr], wt[:, k * FD + wr.start : k * FD + wr.stop], src)
                        nc.tensor.matmul(
                            acc[:], smat[:, (dh + 1) * 128 : (dh + 2) * 128],
                            tz[:], start=False, stop=False, skip_group_check=True,
                        )
                    # dl=+-1 interior ranges
                    tmps = {}
                    for k, (dl, dh, dw) in terms_dlp + terms_dlm:
                        o = -dl * 128 - dw
                        tmp = tpool.tile([128, FD], bf16, name="tmp", tag="tmp")
                        tmps[k] = tmp
                        ir = slice(130, FD) if dl == 1 else slice(0, 382)
                        if dw == 0:
                            src = prevB[pv][:, 130 + o + ir.start : 130 + o + ir.stop]
                        else:
                            src = prevA[pv][:, 129 + o + ir.start : 129 + o + ir.stop]
                        nc.vector.tensor_mul(
                            tmp[:, ir], wt[:, k * FD + ir.start : k * FD + ir.stop], src
                        )
                        nc.tensor.matmul(
                            acc[:, ir], smat[:, (dh + 1) * 128 : (dh + 2) * 128],
                            tmp[:, ir], start=False, stop=False, skip_group_check=True,
                        )

                    # --- halo arrival (two engines so the DMAs overlap) ---
                    ago = ag_out_prev
                    nc.sync.dma_start(
                        prevA[pv][:, 1:129], ago[ds((pid + 7) % 8, 1), :, 128:256]
                    )
                    nc.scalar.dma_start(
                        prevA[pv][:, 641:769], ago[ds((pid2 + 1) % 8, 1), :, 0:128]
                    )
                    # shifted-copy strips over the halo regions (Scalar engine,
                    # keeps DVE free for the boundary multiplies)
                    nc.scalar.copy(prevB[pv][:, 2:130], prevA[pv][:, 1:129])
                    nc.scalar.copy(prevB[pv][:, 642:770], prevA[pv][:, 641:769])

                    # --- boundary stencil work ---
                    last_mm = None
                    for k, (dl, dh, dw) in terms_dlp + terms_dlm:
                        o = -dl * 128 - dw
                        tmp = tmps[k]
                        br = slice(0, 130) if dl == 1 else slice(382, FD)
                        if dw == 0:
                            src = prevB[pv][:, 130 + o + br.start : 130 + o + br.stop]
                        else:
                            src = prevA[pv][:, 129 + o + br.start : 129 + o + br.stop]
                        nc.vector.tensor_mul(
                            tmp[:, br], wt[:, k * FD + br.start : k * FD + br.stop], src
                        )
                        last_mm = nc.tensor.matmul(
                            acc[:, br], smat[:, (dh + 1) * 128 : (dh + 2) * 128],
                            tmp[:, br], start=False, stop=False, skip_group_check=True,
                        )

                    # epilogue, boundary-plane columns first (as one two-block
                    # strided view) so the AllGather staging launches before
                    # the mid columns are done
                    tot = wpool.tile([128, FD], f16, name="tot", tag="tot")
                    inp = wpool.tile([128, FD], f32, name="inp", tag="inp")
                    vp = wpool.tile([128, FD], f32, name="vp", tag="vp")

                    def _bnd(ap):
                        return ap.rearrange("p (b w) -> p b w", b=4)[:, ::3, :]

                    spk_dst = prevA[me][:, 129:641]
                    views = [
                        (
                            _bnd(tot[:, :]), _bnd(acc[:, :]), _bnd(base[:, :]),
                            _bnd(ext[:, :]), _bnd(vd[:, :]), _bnd(inp[:, :]),
                            _bnd(vp[:, :]), _bnd(spk_dst),
                        ),
                        (
                            tot[:, 128:384], acc[:, 128:384], base[:, 128:384],
                            ext[:, 128:384], vd[:, 128:384], inp[:, 128:384],
                            vp[:, 128:384], prevA[me][:, 257:513],
                        ),
                    ]
                    for tot_v, acc_v, base_v, ext_v, vd_v, inp_v, vp_v, spk_v in views:
                        # total = f16(I + 1e-8 * S_all); inp = f32(total) + ext
                        nc.vector.tensor_add(tot_v, acc_v, base_v)
                        nc.vector.tensor_add(inp_v, tot_v, ext_v)
                        # v' = inp*gain + v*decay
                        nc.vector.scalar_tensor_tensor(
                            vp_v, inp_v, GAIN, vd_v, op0=Alu.mult, op1=Alu.add
                        )
                        nc.vector.tensor_scalar(
                            spk_v, vp_v, THRESHOLD, None, op0=Alu.is_ge
                        )
                spikes = prevA[me][:, 129:641]

                # stage boundary planes + AllGather for the next step
                if t < NSTEPS - 1 or rep < repeats - 1:
                    ag_in = dpool.tile([128, 256], bf16, name="ag_in", tag="ag_in")
                    ag_out = dpool.tile(
                        [NCORES, 128, 256], bf16, name="ag_out", tag="ag_out"
                    )
                    src2 = prevA[me][:, 129:641].rearrange(
                        "p (b w) -> p b w", b=4
                    )[:, ::3, :]
                    nc.sync.dma_start(ag_in[:], src2)
                    nc.gpsimd.collective_compute(
                        "AllGather",
                        Alu.bypass,
                        replica_groups=RG,
                        ins=[ag_in.opt()],
                        outs=[ag_out.opt()],
                    )
                    ag_out_prev = ag_out

                # v = v' * (1 - spikes)
                msk = wpool.tile([128, FD], f32, name="msk", tag="msk")
                nc.scalar.activation(
                    msk[:], spikes, mybir.ActivationFunctionType.Identity,
                    bias=1.0, scale=-1.0,
                )
                nc.vector.tensor_mul(vt[me][:], vp[:], msk[:])

                nc.scalar.dma_start(osp_d[t], spikes)
                nc.sync.dma_start(ovl_d[t], vt[me][:])

    nc.compile()
    return nc


def _get_program(repeats=1):
    if repeats not in _PROG:
        _PROG[repeats] = _build_program(repeats=repeats)
    return _PROG[repeats]


def _make_timing_callable(inputs, repeats=1):
    """Build a reusable jitted callable for HW timing (no donation, inputs
    device-resident). Mirrors bass2jax.run_bass_via_pjrt's multi-core path."""
    import sys
    if "/opt/trn_rl_repo" not in sys.path:
        sys.path.insert(0, "/opt/trn_rl_repo")
    import jax
    import concourse.mybir as mybir
    from concourse import bass2jax
    from jax.experimental.shard_map import shard_map
    from jax.sharding import Mesh, NamedSharding, PartitionSpec

    nc = _get_program(repeats=repeats)
    in_maps = _prepare_in_maps(
        inputs["external_input"], inputs["src_ids"], inputs["dst_ids"],
        inputs["values_exp"],
    )
    bass2jax.install_neuronx_cc_hook()

    partition_name = nc.partition_id_tensor.name if nc.partition_id_tensor else None
    in_names, out_names, out_avals, zero_outs = [], [], [], []
    for alloc in nc.m.functions[0].allocations:
        if not isinstance(alloc, mybir.MemoryLocationSet):
            continue
        name = alloc.memorylocations[0].name
        if alloc.kind == "ExternalInput":
            if name != partition_name:
                in_names.append(name)
        elif alloc.kind == "ExternalOutput":
            out_names.append(name)
            shape = tuple(alloc.tensor_shape)
            dtype = mybir.dt.np(alloc.dtype)
            out_avals.append(jax.core.ShapedArray(shape, dtype))
            zero_outs.append(np.zeros(shape, dtype))
    n_params = len(in_names)
    all_in_names = list(in_names) + list(out_names)
    if partition_name is not None:
        all_in_names.append(partition_name)

    def _body(*args):
        operands = list(args)
        if partition_name is not None:
            operands.append(bass2jax.partition_id_tensor())
        outs = bass2jax._bass_exec_p.bind(
            *operands,
            out_avals=tuple(out_avals),
            in_names=tuple(all_in_names),
            out_names=tuple(out_names),
            lowering_input_output_aliases=(),
            sim_require_finite=True,
            sim_require_nnan=True,
            nc=nc,
        )
        return tuple(outs)

    devices = jax.devices()[:NCORES]
    mesh = Mesh(np.asarray(devices), ("core",))
    n_outs = len(out_names)
    in_specs = (PartitionSpec("core"),) * (n_params + n_outs)
    out_specs = (PartitionSpec("core"),) * n_outs
    fn = jax.jit(
        shard_map(_body, mesh=mesh, in_specs=in_specs, out_specs=out_specs,
                  check_rep=False),
        keep_unused=True,
    )
    sharding = NamedSharding(mesh, PartitionSpec("core"))
    dev_args = []
    for i, name in enumerate(in_names):
        concat = np.concatenate(
            [np.asarray(in_maps[c][name]) for c in range(NCORES)], axis=0
        )
        dev_args.append(jax.device_put(concat, sharding))
    for z in zero_outs:
        zz = np.zeros((NCORES * z.shape[0], *z.shape[1:]), z.dtype)
        dev_args.append(jax.device_put(zz, sharding))

    return fn, (lambda i: dev_args)


def _core_view(vol):
    """[L,H,W] volume -> [NCORES, 128, FD] in (core, h, l_local*W + w) layout."""
    v = vol.reshape(NCORES, SLAB, H, W).transpose(0, 2, 1, 3)
    return np.ascontiguousarray(v.reshape(NCORES, H, FD))


def _prepare_in_maps(external_input, src_ids, dst_ids, values_exp):
    import ml_dtypes

    src = np.asarray(src_ids).astype(np.int64)
    dst = np.asarray(dst_ids).astype(np.int64)
    exp = np.asarray(values_exp).astype(np.int64)
    ext = np.asarray(external_input).astype(np.float32)

    delta = dst - src
    wvals = (1 << exp).astype(np.float32)

    dvals = {dl * (H * W) + dh * W + dw: k for k, (dl, dh, dw) in enumerate(OFFSETS)}
    uniq = np.unique(delta)
    unknown = [d for d in uniq.tolist() if d not in dvals]
    if unknown:
        raise ValueError(f"unexpected edge deltas (not a 26-stencil): {unknown[:10]}")

    S_all = np.zeros(N, np.float64)
    wt = np.zeros((NCORES, H, 26, FD), np.float32)
    for k, (dl, dh, dw) in enumerate(OFFSETS):
        dval = dl * (H * W) + dh * W + dw
        m = delta == dval
        plane = np.zeros(N, np.float32)
        plane[dst[m]] = wvals[m]
        S_all += plane
        vol = plane.reshape(L, H, W)
        # W'_k[h, f] = W_k[h + dh, f]  (zero beyond the h boundary)
        sh = np.zeros_like(vol)
        if dh >= 0:
            sh[:, : H - dh, :] = vol[:, dh:, :]
        else:
            sh[:, -dh:, :] = vol[:, : H + dh, :]
        per_core = sh.reshape(NCORES, SLAB, H, W).transpose(0, 2, 1, 3)
        wt[:, :, k, :] = per_core.reshape(NCORES, H, FD)

    wt_bf = wt.reshape(NCORES, H, 26 * FD).astype(ml_dtypes.bfloat16)
    base_c = _core_view((S_all * 1e-8).astype(np.float32).reshape(L, H, W))
    ext_c = _core_view(ext.reshape(L, H, W))

    smat = np.zeros((128, 3, 128), np.float32)
    for j, dh in enumerate((-1, 0, 1)):
        for h in range(128):
            h2 = h + dh
            if 0 <= h2 < 128:
                smat[h, j, h2] = 1.0
    smat_bf = smat.reshape(128, 3 * 128).astype(ml_dtypes.bfloat16)

    in_maps = []
    for c in range(NCORES):
        in_maps.append(
            {
                "wt": np.ascontiguousarray(wt_bf[c]),
                "smat": smat_bf,
                "base": np.ascontiguousarray(base_c[c]),
                "ext": np.ascontiguousarray(ext_c[c]),
            }
        )
    return in_maps


def _assemble(results):
    """Per-core [NSTEPS,128,FD] outputs -> full [NSTEPS, N] arrays."""
    def full(key):
        arr = np.stack(
            [np.asarray(results[c][key]).astype(np.float32) for c in range(NCORES)]
        )
        # [c, t, h, l_local, w] -> [t, c, l_local, h, w]
        arr = arr.reshape(NCORES, NSTEPS, H, SLAB, W).transpose(1, 0, 3, 2, 4)
        return np.ascontiguousarray(arr.reshape(NSTEPS, N))

    return full("out_spikes"), full("out_volts")


LAST_RESULTS = None


def kernel(external_input, src_ids, dst_ids, values_exp, num_steps):
    global LAST_RESULTS
    assert int(num_steps) == NSTEPS, f"kernel compiled for {NSTEPS} steps"
    ext = np.asarray(external_input)
    assert ext.shape == (N,)
    assert np.asarray(src_ids).shape[0] == np.asarray(dst_ids).shape[0]

    import sys
    if "/opt/trn_rl_repo" not in sys.path:
        sys.path.insert(0, "/opt/trn_rl_repo")
    from concourse import bass_utils

    nc = _get_program()
    in_maps = _prepare_in_maps(external_input, src_ids, dst_ids, values_exp)
    res = bass_utils.run_bass_kernel_spmd(nc, in_maps, core_ids=list(range(NCORES)))
    LAST_RESULTS = res
    spikes, volts = _assemble(res.results)
    ve = np.asarray(values_exp)
    return spikes, volts, ve


# revision 35
# speedup vs baseline: 1.0527x; 1.0527x over previous
"""Trainium2 Bass kernel for the Brain3DNetwork LIF spiking network.

Problem: N = 32*128*128 neurons on a 3D grid, 26-neighbor stencil edges in
COO form with weights 2^exp (exp in 0..8), 10 LIF timesteps with a
log-domain weighted gather-scatter (mathematically: per-dst sum of
w * (prev_spike + 1e-8), computed in f32, cast to f16), then LIF dynamics.

Strategy:
  * Shard the L dimension (32 planes) across 8 NeuronCores, 4 planes each.
  * SBUF layout per core: [h=128 partitions, (l_local, w) = 512 free].
  * The COO edge list is a regular 26-offset stencil; host preprocessing
    groups edges by (dl,dh,dw) offset into 26 dense weight planes per core.
  * Per step, per offset: one bf16 elementwise multiply on the Vector engine
    (weights x shifted-spikes view; shifts along l/w are free-dim AP offsets,
    with a 1-element-shifted spike copy to keep all views 4-byte aligned),
    then a shift-matrix matmul on the Tensor engine accumulating the
    h-shifted sum into PSUM in f32 (integer-exact).
  * Since f32(1 + 1e-8) == 1.0 exactly, the reference's w*(prev+1e-8) equals
    (integer spike contribution) + 1e-8*(sum of non-spiking weights); adding
    a precomputed base = 1e-8 * (total incoming weight) instead differs by
    1e-8*I which is always far below f16 rounding granularity.
  * Cross-core halo exchange of boundary spike planes via an 8-core
    AllGather each step; per-core neighbor slices are addressed dynamically
    with partition_id and out-of-range DMAs are skipped (edge cores).
  * The early-exit (prev.sum()==0 -> total=0) is exact at step 0 (handled by
    construction: step 0 skips the stencil entirely) and unreachable
    afterwards (~46% of neurons spike at step 0 from the external drive).
"""

import numpy as np

L, H, W = 32, 128, 128
N = L * H * W
NCORES = 8
SLAB = L // NCORES          # 4 l-planes per core
FD = SLAB * W               # 512 free dim of the working set
PF = 772                    # prevA free dim: 1 pad + 128 halo + 512 local + 128 halo + 3 pad
NSTEPS = 10
E_EDGES = 13192568

TAU, THRESHOLD = 30.0, 0.1
DECAY = float(np.float32(np.exp(-1.0 / TAU)))
GAIN = float(np.float32(TAU * (1.0 - np.exp(-1.0 / TAU))))

OFFSETS = [
    (dl, dh, dw)
    for dl in (-1, 0, 1)
    for dh in (-1, 0, 1)
    for dw in (-1, 0, 1)
    if not (dl == 0 and dh == 0 and dw == 0)
]

_PROG = {}  # cached compiled programs, keyed by repeats


def _build_program(repeats=1):
    import sys
    if "/opt/trn_rl_repo" not in sys.path:
        sys.path.insert(0, "/opt/trn_rl_repo")
    import concourse.bass as bass
    import concourse.mybir as mybir
    import concourse.tile as tile
    from concourse import bacc
    from concourse.bass import ds

    f32 = mybir.dt.float32
    f16 = mybir.dt.float16
    bf16 = mybir.dt.bfloat16
    Alu = mybir.AluOpType

    nc = bacc.Bacc(
        "TRN2",
        target_bir_lowering=False,
        debug=False,
        enable_asserts=False,
        num_devices=NCORES,
    )

    wt_d = nc.dram_tensor("wt", [128, 26 * FD], bf16, kind="ExternalInput")
    smat_d = nc.dram_tensor("smat", [128, 3 * 128], bf16, kind="ExternalInput")
    base_d = nc.dram_tensor("base", [128, FD], f32, kind="ExternalInput")
    ext_d = nc.dram_tensor("ext", [128, FD], f32, kind="ExternalInput")
    osp_d = nc.dram_tensor("out_spikes", [NSTEPS, 128, FD], bf16, kind="ExternalOutput")
    ovl_d = nc.dram_tensor("out_volts", [NSTEPS, 128, FD], f32, kind="ExternalOutput")

    RG = [list(range(NCORES))]

    with tile.TileContext(nc) as tc:
        with (
            tc.tile_pool(name="const", bufs=1) as cpool,
            tc.tile_pool(name="state", bufs=1) as spool,
            tc.tile_pool(name="work", bufs=2) as wpool,
            tc.tile_pool(name="tmpp", bufs=20) as tpool,
            tc.tile_pool(name="psum", bufs=2, space="PSUM") as ppool,
            tc.tile_pool(name="dram", bufs=2, space="DRAM") as dpool,
        ):
            # ext first: step 0 needs only it
            ext = cpool.tile([128, FD], f32, name="ext_s")
            nc.sync.dma_start(ext[:], ext_d[:])
            smat = cpool.tile([128, 3 * 128], bf16, name="smat_s")
            nc.scalar.dma_start(smat[:], smat_d[:])
            base = cpool.tile([128, FD], f32, name="base_s")
            nc.scalar.dma_start(base[:], base_d[:])
            # weights: split across queues/engines to parallelize the 3.4MB load
            wt = cpool.tile([128, 26 * FD], bf16, name="wt_s")
            NCH = 8
            cw = 26 * FD // NCH
            for j in range(NCH):
                eng = nc.sync if j % 2 == 0 else nc.scalar
                eng.dma_start(wt[:, j * cw : (j + 1) * cw], wt_d[:, j * cw : (j + 1) * cw])

            prevA = [spool.tile([128, PF], bf16, name=f"prevA{i}") for i in range(2)]
            prevB = [spool.tile([128, PF], bf16, name=f"prevB{i}") for i in range(2)]
            vt = [spool.tile([128, FD], f32, name=f"vt{i}") for i in range(2)]
            for i in range(2):
                nc.vector.memset(prevA[i][:], 0.0)
            # persistent dl=0 product tiles; the never-written edge column of
            # each group stays zero from this memset (its weight is zero, and
            # skipping it removes the halo-column read dependency entirely).
            tmpza = [spool.tile([128, FD], bf16, name=f"tmpza{i}") for i in range(2)]
            tmpzb = [spool.tile([128, FD], bf16, name=f"tmpzb{i}") for i in range(2)]
            for z in tmpza + tmpzb:
                nc.vector.memset(z[:], 0.0)

            pid = nc.sync.partition_id()
            pid2 = nc.scalar.partition_id()

            # stencil terms grouped: dl=0 first (full-width, no halo deps),
            # then dl=+-1 split into interior/boundary column ranges.
            terms_dl0 = [(k, o) for k, o in enumerate(OFFSETS) if o[0] == 0]
            terms_dlp = [(k, o) for k, o in enumerate(OFFSETS) if o[0] == 1]
            terms_dlm = [(k, o) for k, o in enumerate(OFFSETS) if o[0] == -1]

            ag_out_prev = None
            for rep in range(repeats):
              for t in range(NSTEPS):
                me = t % 2
                pv = 1 - me

                if t == 0:
                    vp = wpool.tile([128, FD], f32, name="vp", tag="vp")
                    nc.vector.tensor_scalar(vp[:], ext[:], GAIN, None, op0=Alu.mult)
                else:
                    acc = ppool.tile([128, FD], f32, name="acc", tag="acc")
                    # zero PSUM so matmuls can accumulate in any order
                    # (no start=True ordering hazard)
                    nc.vector.memset(acc[:], 0.0)
                    # v*decay on the scalar engine, off the critical path
                    vd = wpool.tile([128, FD], f32, name="vd", tag="vd")
                    nc.scalar.mul(vd[:], vt[pv][:], DECAY)
                    # interior part of the 1-element-shifted spike copy
                    # (keeps dw=0 views 4B-aligned); halo strips come later
                    nc.vector.tensor_copy(
                        prevB[pv][:, 130:642], prevA[pv][:, 129:641]
                    )

                    # --- interior stencil work (no halo dependency) ---
                    # dl=0: one edge column per term is out-of-plane (zero
                    # weight); skip it so the mul never reads a halo column.
                    zi_a = zi_b = 0
                    for k, (dl, dh, dw) in terms_dl0:
                        o = -dw
                        if dw == 0:
                            # reads prevB[130:642) == local data only: full width
                            tz = tpool.tile([128, FD], bf16, name="tmp", tag="tmp")
                            wr = slice(0, FD)
                        elif dw < 0:
                            tz = tmpza[zi_a % 2]; zi_a += 1
                            wr = slice(0, FD - 1)      # col FD-1 stays zero
                        else:
                            tz = tmpzb[zi_b % 2]; zi_b += 1
                            wr = slice(1, FD)          # col 0 stays zero
                        if dw == 0:
                            src = prevB[pv][:, 130 + o + wr.start : 130 + o + wr.stop]
                        else:
                            src = prevA[pv][:, 129 + o + wr.start : 129 + o + wr.stop]
                        nc.vector.tensor_mul(tz[:, wr], wt[:, k * FD + wr.start : k * FD + wr.stop], src)
                        nc.tensor.matmul(
                            acc[:], smat[:, (dh + 1) * 128 : (dh + 2) * 128],
                            tz[:], start=False, stop=False, skip_group_check=True,
                        )
                    # dl=+-1 interior ranges
                    tmps = {}
                    for k, (dl, dh, dw) in terms_dlp + terms_dlm:
                        o = -dl * 128 - dw
                        tmp = tpool.tile([128, FD], bf16, name="tmp", tag="tmp")
                        tmps[k] = tmp
                        ir = slice(130, FD) if dl == 1 else slice(0, 382)
                        if dw == 0:
                            src = prevB[pv][:, 130 + o + ir.start : 130 + o + ir.stop]
                        else:
                            src = prevA[pv][:, 129 + o + ir.start : 129 + o + ir.stop]
                        nc.vector.tensor_mul(
                            tmp[:, ir], wt[:, k * FD + ir.start : k * FD + ir.stop], src
                        )
                        nc.tensor.matmul(
                            acc[:, ir], smat[:, (dh + 1) * 128 : (dh + 2) * 128],
                            tmp[:, ir], start=False, stop=False, skip_group_check=True,
                        )

                    # --- halo arrival (two engines so the DMAs overlap) ---
                    ago = ag_out_prev
                    nc.sync.dma_start(
                        prevA[pv][:, 1:129], ago[ds((pid + 7) % 8, 1), :, 128:256]
                    )
                    nc.scalar.dma_start(
                        prevA[pv][:, 641:769], ago[ds((pid2 + 1) % 8, 1), :, 0:128]
                    )
                    # shifted-copy strips over the halo regions (Scalar engine,
                    # keeps DVE free for the boundary multiplies)
                    nc.scalar.copy(prevB[pv][:, 2:130], prevA[pv][:, 1:129])
                    nc.scalar.copy(prevB[pv][:, 642:770], prevA[pv][:, 641:769])

                    # --- boundary stencil work ---
                    last_mm = None
                    for k, (dl, dh, dw) in terms_dlp + terms_dlm:
                        o = -dl * 128 - dw
                        tmp = tmps[k]
                        br = slice(0, 130) if dl == 1 else slice(382, FD)
                        if dw == 0:
                            src = prevB[pv][:, 130 + o + br.start : 130 + o + br.stop]
                        else:
                            src = prevA[pv][:, 129 + o + br.start : 129 + o + br.stop]
                        nc.vector.tensor_mul(
                            tmp[:, br], wt[:, k * FD + br.start : k * FD + br.stop], src
                        )
                        last_mm = nc.tensor.matmul(
                            acc[:, br], smat[:, (dh + 1) * 128 : (dh + 2) * 128],
                            tmp[:, br], start=False, stop=False, skip_group_check=True,
                        )

                    # epilogue, boundary-plane columns first (as one two-block
                    # strided view) so the AllGather staging launches before
                    # the mid columns are done
                    tot = wpool.tile([128, FD], f16, name="tot", tag="tot")
                    inp = wpool.tile([128, FD], f32, name="inp", tag="inp")
                    vp = wpool.tile([128, FD], f32, name="vp", tag="vp")

                    def _bnd(ap):
                        return ap.rearrange("p (b w) -> p b w", b=4)[:, ::3, :]

                    spk_dst = prevA[me][:, 129:641]
                    views = [
                        (
                            _bnd(tot[:, :]), _bnd(acc[:, :]), _bnd(base[:, :]),
                            _bnd(ext[:, :]), _bnd(vd[:, :]), _bnd(inp[:, :]),
                            _bnd(vp[:, :]), _bnd(spk_dst),
                        ),
                        (
                            tot[:, 128:384], acc[:, 128:384], base[:, 128:384],
                            ext[:, 128:384], vd[:, 128:384], inp[:, 128:384],
                            vp[:, 128:384], prevA[me][:, 257:513],
                        ),
                    ]
                    for tot_v, acc_v, base_v, ext_v, vd_v, inp_v, vp_v, spk_v in views:
                        # total = f16(I + 1e-8 * S_all); inp = f32(total) + ext
                        nc.vector.tensor_add(tot_v, acc_v, base_v)
                        nc.vector.tensor_add(inp_v, tot_v, ext_v)
                        # v' = inp*gain + v*decay
                        nc.vector.scalar_tensor_tensor(
                            vp_v, inp_v, GAIN, vd_v, op0=Alu.mult, op1=Alu.add
                        )
                        nc.vector.tensor_scalar(
                            spk_v, vp_v, THRESHOLD, None, op0=Alu.is_ge
                        )
                spikes = prevA[me][:, 129:641]

                if t == 0:
                    nc.vector.tensor_scalar(
                        spikes, vp[:], THRESHOLD, None, op0=Alu.is_ge
                    )

                # stage boundary planes + AllGather for the next step
                if t < NSTEPS - 1 or rep < repeats - 1:
                    ag_in = dpool.tile([128, 256], bf16, name="ag_in", tag="ag_in")
                    ag_out = dpool.tile(
                        [NCORES, 128, 256], bf16, name="ag_out", tag="ag_out"
                    )
                    src2 = prevA[me][:, 129:641].rearrange(
                        "p (b w) -> p b w", b=4
                    )[:, ::3, :]
                    nc.sync.dma_start(ag_in[:], src2)
                    nc.gpsimd.collective_compute(
                        "AllGather",
                        Alu.bypass,
                        replica_groups=RG,
                        ins=[ag_in.opt()],
                        outs=[ag_out.opt()],
                    )
                    ag_out_prev = ag_out

                # v = v' * (1 - spikes)
                msk = wpool.tile([128, FD], f32, name="msk", tag="msk")
                nc.scalar.activation(
                    msk[:], spikes, mybir.ActivationFunctionType.Identity,
                    bias=1.0, scale=-1.0,
                )
                nc.vector.tensor_mul(vt[me][:], vp[:], msk[:])

                nc.scalar.dma_start(osp_d[t], spikes)
                nc.sync.dma_start(ovl_d[t], vt[me][:])

    nc.compile()
    return nc


def _get_program(repeats=1):
    if repeats not in _PROG:
        _PROG[repeats] = _build_program(repeats=repeats)
    return _PROG[repeats]


def _make_timing_callable(inputs, repeats=1):
    """Build a reusable jitted callable for HW timing (no donation, inputs
    device-resident). Mirrors bass2jax.run_bass_via_pjrt's multi-core path."""
    import sys
    if "/opt/trn_rl_repo" not in sys.path:
        sys.path.insert(0, "/opt/trn_rl_repo")
    import jax
    import concourse.mybir as mybir
    from concourse import bass2jax
    from jax.experimental.shard_map import shard_map
    from jax.sharding import Mesh, NamedSharding, PartitionSpec

    nc = _get_program(repeats=repeats)
    in_maps = _prepare_in_maps(
        inputs["external_input"], inputs["src_ids"], inputs["dst_ids"],
        inputs["values_exp"],
    )
    bass2jax.install_neuronx_cc_hook()

    partition_name = nc.partition_id_tensor.name if nc.partition_id_tensor else None
    in_names, out_names, out_avals, zero_outs = [], [], [], []
    for alloc in nc.m.functions[0].allocations:
        if not isinstance(alloc, mybir.MemoryLocationSet):
            continue
        name = alloc.memorylocations[0].name
        if alloc.kind == "ExternalInput":
            if name != partition_name:
                in_names.append(name)
        elif alloc.kind == "ExternalOutput":
            out_names.append(name)
            shape = tuple(alloc.tensor_shape)
            dtype = mybir.dt.np(alloc.dtype)
            out_avals.append(jax.core.ShapedArray(shape, dtype))
            zero_outs.append(np.zeros(shape, dtype))
    n_params = len(in_names)
    all_in_names = list(in_names) + list(out_names)
    if partition_name is not None:
        all_in_names.append(partition_name)

    def _body(*args):
        operands = list(args)
        if partition_name is not None:
            operands.append(bass2jax.partition_id_tensor())
        outs = bass2jax._bass_exec_p.bind(
            *operands,
            out_avals=tuple(out_avals),
            in_names=tuple(all_in_names),
            out_names=tuple(out_names),
            lowering_input_output_aliases=(),
            sim_require_finite=True,
            sim_require_nnan=True,
            nc=nc,
        )
        return tuple(outs)

    devices = jax.devices()[:NCORES]
    mesh = Mesh(np.asarray(devices), ("core",))
    n_outs = len(out_names)
    in_specs = (PartitionSpec("core"),) * (n_params + n_outs)
    out_specs = (PartitionSpec("core"),) * n_outs
    fn = jax.jit(
        shard_map(_body, mesh=mesh, in_specs=in_specs, out_specs=out_specs,
                  check_rep=False),
        keep_unused=True,
    )
    sharding = NamedSharding(mesh, PartitionSpec("core"))
    dev_args = []
    for i, name in enumerate(in_names):
        concat = np.concatenate(
            [np.asarray(in_maps[c][name]) for c in range(NCORES)], axis=0
        )
        dev_args.append(jax.device_put(concat, sharding))
    for z in zero_outs:
        zz = np.zeros((NCORES * z.shape[0], *z.shape[1:]), z.dtype)
        dev_args.append(jax.device_put(zz, sharding))

    return fn, (lambda i: dev_args)


def _core_view(vol):
    """[L,H,W] volume -> [NCORES, 128, FD] in (core, h, l_local*W + w) layout."""
    v = vol.reshape(NCORES, SLAB, H, W).transpose(0, 2, 1, 3)
    return np.ascontiguousarray(v.reshape(NCORES, H, FD))


def _prepare_in_maps(external_input, src_ids, dst_ids, values_exp):
    import ml_dtypes

    src = np.asarray(src_ids).astype(np.int64)
    dst = np.asarray(dst_ids).astype(np.int64)
    exp = np.asarray(values_exp).astype(np.int64)
    ext = np.asarray(external_input).astype(np.float32)

    delta = dst - src
    wvals = (1 << exp).astype(np.float32)

    dvals = {dl * (H * W) + dh * W + dw: k for k, (dl, dh, dw) in enumerate(OFFSETS)}
    uniq = np.unique(delta)
    unknown = [d for d in uniq.tolist() if d not in dvals]
    if unknown:
        raise ValueError(f"unexpected edge deltas (not a 26-stencil): {unknown[:10]}")

    S_all = np.zeros(N, np.float64)
    wt = np.zeros((NCORES, H, 26, FD), np.float32)
    for k, (dl, dh, dw) in enumerate(OFFSETS):
        dval = dl * (H * W) + dh * W + dw
        m = delta == dval
        plane = np.zeros(N, np.float32)
        plane[dst[m]] = wvals[m]
        S_all += plane
        vol = plane.reshape(L, H, W)
        # W'_k[h, f] = W_k[h + dh, f]  (zero beyond the h boundary)
        sh = np.zeros_like(vol)
        if dh >= 0:
            sh[:, : H - dh, :] = vol[:, dh:, :]
        else:
            sh[:, -dh:, :] = vol[:, : H + dh, :]
        per_core = sh.reshape(NCORES, SLAB, H, W).transpose(0, 2, 1, 3)
        wt[:, :, k, :] = per_core.reshape(NCORES, H, FD)

    wt_bf = wt.reshape(NCORES, H, 26 * FD).astype(ml_dtypes.bfloat16)
    base_c = _core_view((S_all * 1e-8).astype(np.float32).reshape(L, H, W))
    ext_c = _core_view(ext.reshape(L, H, W))

    smat = np.zeros((128, 3, 128), np.float32)
    for j, dh in enumerate((-1, 0, 1)):
        for h in range(128):
            h2 = h + dh
            if 0 <= h2 < 128:
                smat[h, j, h2] = 1.0
    smat_bf = smat.reshape(128, 3 * 128).astype(ml_dtypes.bfloat16)

    in_maps = []
    for c in range(NCORES):
        in_maps.append(
            {
                "wt": np.ascontiguousarray(wt_bf[c]),
                "smat": smat_bf,
                "base": np.ascontiguousarray(base_c[c]),
                "ext": np.ascontiguousarray(ext_c[c]),
            }
        )
    return in_maps


def _assemble(results):
    """Per-core [NSTEPS,128,FD] outputs -> full [NSTEPS, N] arrays."""
    def full(key):
        arr = np.stack(
            [np.asarray(results[c][key]).astype(np.float32) for c in range(NCORES)]
        )
        # [c, t, h, l_local, w] -> [t, c, l_local, h, w]
        arr = arr.reshape(NCORES, NSTEPS, H, SLAB, W).transpose(1, 0, 3, 2, 4)
        return np.ascontiguousarray(arr.reshape(NSTEPS, N))

    return full("out_spikes"), full("out_volts")


LAST_RESULTS = None


def kernel(external_input, src_ids, dst_ids, values_exp, num_steps):
    global LAST_RESULTS
    assert int(num_steps) == NSTEPS, f"kernel compiled for {NSTEPS} steps"
    ext = np.asarray(external_input)
    assert ext.shape == (N,)
    assert np.asarray(src_ids).shape[0] == np.asarray(dst_ids).shape[0]

    import sys
    if "/opt/trn_rl_repo" not in sys.path:
        sys.path.insert(0, "/opt/trn_rl_repo")
    from concourse import bass_utils

    nc = _get_program()
    in_maps = _prepare_in_maps(external_input, src_ids, dst_ids, values_exp)
    res = bass_utils.run_bass_kernel_spmd(nc, in_maps, core_ids=list(range(NCORES)))
    LAST_RESULTS = res
    spikes, volts = _assemble(res.results)
    ve = np.asarray(values_exp)
    return spikes, volts, ve
